# revision 11
# baseline (speedup 1.0000x reference)
"""AGNN layer (cosine-attention message passing) on 8 TRN2 NeuronCores.

Host sharding: append self-loops, sort edges by destination node, cut the node
range into blocks (<=128 nodes, bounded edge count), hand contiguous block
runs to the 8 cores. Every softmax segment then lives on one core: no
collectives anywhere.

v3 device kernel ("no dst gather"): only the per-edge SRC rows are gathered
(normalized x, bf16, 256B rows — halves SWDGE descriptor+byte traffic vs v1),
one big SWDGE call per table per block. Each gathered group of 8 tiles is
transposed SBUF->SBUF by the DMA XBAR (dma_start_transpose, round-robined
over the sync/scalar/vector HWDGE queues); the Tensor engine then computes
dots[e, n] = xnT_tile^T @ xdT_block against an SBUF-resident per-block
transposed dst table. ACT exponentiates whole groups (scale=beta, constant).
DVE builds the weight matrix in 3 block-wide batched passes:
pw2 = is_equal(iota, dstrel) * exp(beta cos) * ||x_src||, so the two
PSUM-accumulating matmuls per tile yield sum(w * x_src) (norms fold raw x
back out of the normalized rows) and sum(w) (rhs = 1/||x_src||).

Logits are cosines scaled by beta (bounded), so exp never overflows and the
reference's segment-max subtraction cancels exactly -- single pass suffices.
"""

import numpy as np
import ml_dtypes

import concourse.bacc as bacc
import concourse.mybir as mybir
import concourse.tile as tile
import concourse.tile_sem_assignment as _tsa

# Tile's DMASW-lane round-robin is SWDGE-queue-oblivious; a completion sem
# shared by two queues desyncs the ucode's per-queue ring-reclaim waits.
# Partition the 8 lanes so queue q owns lanes {2q, 2q+1}.
_orig_assign_tick = _tsa.TileClockTick._assign_tick


def _assign_tick_queue_aware(self, inst):
    if (
        isinstance(inst, mybir.InstDMAGatherAnt)
        and inst.engine == mybir.EngineType.Pool
    ):
        qn = inst.queue_num
        if not hasattr(self, "_qcnt"):
            self._qcnt = {}
        cnt = self._qcnt.get(qn, 0)
        self._qcnt[qn] = cnt + 1
        self.next_sw_dma_idx = qn * 2 + (cnt % 2)
    return _orig_assign_tick(self, inst)


_tsa.TileClockTick._assign_tick = _assign_tick_queue_aware

P = 128
N_NODES = 50000
D_FEAT = 128
NCORES = 8
HI_BASE = 32768  # int16 gather index limit
TPB_LO = 23  # tiles per block for src<HI_BASE edges
TPB_HI = 12  # tiles per block for src>=HI_BASE edges
GROUP = 8  # tiles per transpose/exp batch
BLK_SENTINEL = 300.0

F32 = mybir.dt.float32
BF16 = mybir.dt.bfloat16
I16 = mybir.dt.int16


def _build_graph(N, D, NB, tpb_lo, tpb_hi, hi_base):
    tpb = tpb_lo + tpb_hi
    gl, gh = tpb_lo * 8, tpb_hi * 8  # idx cols (16-wrapped)
    ga = gl + gh
    nc = bacc.Bacc(
        "TRN2", target_bir_lowering=False, debug=False, enable_asserts=False,
        num_swdge_queues=4,
    )
    n_lo = min(hi_base, N)
    n_hi = max(N - hi_base, 8)
    xb_lo = nc.dram_tensor("xb_lo", [n_lo, D], BF16, kind="ExternalInput").ap()
    xb_hi = nc.dram_tensor("xb_hi", [n_hi, D], BF16, kind="ExternalInput").ap()
    idx_all = nc.dram_tensor("idx_all", [P, NB * ga], I16, kind="ExternalInput").ap()
    dblk = nc.dram_tensor("dblk", [P, NB * tpb], BF16, kind="ExternalInput").ap()
    normt = nc.dram_tensor("normt", [P, NB * tpb], BF16, kind="ExternalInput").ap()
    rcol = nc.dram_tensor("rcol", [P, NB * tpb], BF16, kind="ExternalInput").ap()
    xdT = nc.dram_tensor("xdT", [P, NB * P], BF16, kind="ExternalInput").ap()
    beta128 = nc.dram_tensor("beta128", [P, 1], F32, kind="ExternalInput").ap()
    out_ext = nc.dram_tensor("out", [NB * P, D], F32, kind="ExternalOutput").ap()

    # lo/hi-aligned transpose groups: [t0, t1) ranges that never straddle
    groups = []
    for t0 in range(0, tpb_lo, GROUP):
        groups.append((t0, min(t0 + GROUP, tpb_lo)))
    for t0 in range(tpb_lo, tpb, GROUP):
        groups.append((t0, min(t0 + GROUP, tpb)))

    with tile.TileContext(nc) as tc:
        with (
            tc.tile_pool(name="const", bufs=1) as constp,
            tc.tile_pool(name="idx", bufs=3) as idxp,
            tc.tile_pool(name="meta", bufs=3) as metap,
            tc.tile_pool(name="gsl", bufs=3) as gslp,
            tc.tile_pool(name="gsh", bufs=3) as gshp,
            tc.tile_pool(name="xnt", bufs=2) as xntp,
            tc.tile_pool(name="expd", bufs=2) as expdp,
            tc.tile_pool(name="pw", bufs=2) as pwp,
            tc.tile_pool(name="orow", bufs=2) as orowp,
            tc.tile_pool(name="cols", bufs=4) as colp,
            tc.tile_pool(name="dots", bufs=2, space="PSUM") as dotsp,
            tc.tile_pool(name="acc", bufs=2, space="PSUM") as accp,
        ):
            # ---- constants ----
            iota_i16 = constp.tile([P, P], I16)
            nc.gpsimd.iota(iota_i16[:], pattern=[[1, P]], base=0, channel_multiplier=0)
            iota_bf = constp.tile([P, P], BF16)
            nc.vector.tensor_copy(iota_bf[:], iota_i16[:])
            beta_sb = constp.tile([P, 1], F32)
            nc.sync.dma_start(out=beta_sb[:], in_=beta128[:, :])
            # resident per-block transposed dst tables (12.8KB/partition)
            xdT_sb = constp.tile([P, NB * P], BF16)
            nc.sync.dma_start(out=xdT_sb[:], in_=xdT[:, :])

            hw_eng = [nc.sync, nc.scalar]
            self_q = [0, 0]  # swdge queue rr, hwdge engine rr
            for b in range(NB):
                idxt = idxp.tile([P, ga], I16, tag="idxt")
                nc.sync.dma_start(out=idxt[:], in_=idx_all[:, b * ga : (b + 1) * ga])
                dbt = metap.tile([P, tpb, 1], BF16, tag="dbt")
                nc.sync.dma_start(out=dbt[:], in_=dblk[:, b * tpb : (b + 1) * tpb])
                nrt = metap.tile([P, tpb, 1], BF16, tag="nrt")
                nc.sync.dma_start(out=nrt[:], in_=normt[:, b * tpb : (b + 1) * tpb])
                rct = metap.tile([P, tpb], BF16, tag="rct")
                nc.sync.dma_start(out=rct[:], in_=rcol[:, b * tpb : (b + 1) * tpb])

                def gather_rows(out_tile, tab_ap, idx_tile, total):
                    # ucode ring holds 128 in-flight descs/engine -> <=1024 rows
                    off = 0
                    while off < total:
                        ni = min(1024, total - off)
                        nc.gpsimd.dma_gather(
                            out_tile[:, off // P : (off + ni) // P, :],
                            tab_ap,
                            idx_tile[:, off // 16 : (off + ni) // 16],
                            ni, ni, D,
                            queue_num=self_q[0] % 4,
                        )
                        self_q[0] += 1
                        off += ni

                xs_lo = gslp.tile([P, tpb_lo, D], BF16, tag="xsl")
                gather_rows(xs_lo, xb_lo[:, :], idxt[:, 0:gl], tpb_lo * P)
                xs_hi = gshp.tile([P, tpb_hi, D], BF16, tag="xsh")
                gather_rows(xs_hi, xb_hi[:, :], idxt[:, gl:ga], tpb_hi * P)

                def xs_slice(t0, t1):
                    if t1 <= tpb_lo:
                        return xs_lo[:, t0:t1, :]
                    return xs_hi[:, t0 - tpb_lo : t1 - tpb_lo, :]

                acc_ps = accp.tile([P, D + 1], F32, tag="acc")
                xd_blk = xdT_sb[:, b * P : (b + 1) * P]
                xnT = xntp.tile([P, tpb, P], BF16, tag="xnT")
                expd = expdp.tile([P, tpb, P], BF16, tag="expd")

                for (t0, t1) in groups:
                    gsz = t1 - t0
                    # DMA XBAR: SBUF->SBUF per-tile transposes for the group
                    eng = hw_eng[self_q[1] % 2]
                    self_q[1] += 1
                    eng.dma_start(
                        out=xnT[:, t0:t1, :], in_=xs_slice(t0, t1), transpose=True
                    )
                    # PE: dots[e, n] over the block's node range
                    dots = dotsp.tile([P, GROUP, P], F32, tag="dots")
                    for t in range(t0, t1):
                        nc.tensor.matmul(
                            dots[:, t - t0, :], lhsT=xnT[:, t, :], rhs=xd_blk,
                            start=True, stop=True, skip_group_check=True,
                        )
                    # ACT: batched exp(beta * cos)
                    nc.scalar.activation(
                        out=expd[:, t0:t1, :], in_=dots[:, 0:gsz, :],
                        func=mybir.ActivationFunctionType.Exp, scale=beta_sb[:],
                    )

                # DVE: block-wide batched weight build
                oh = pwp.tile([P, tpb, P], BF16, tag="oh")
                nc.vector.tensor_tensor(
                    out=oh[:], in0=iota_bf[:][:, None, :].to_broadcast([P, tpb, P]),
                    in1=dbt[:].to_broadcast([P, tpb, P]),
                    op=mybir.AluOpType.is_equal,
                )
                nc.vector.tensor_tensor(
                    out=oh[:], in0=oh[:], in1=expd[:],
                    op=mybir.AluOpType.mult,
                )
                pw2 = pwp.tile([P, tpb, P], BF16, tag="pw2")
                nc.vector.tensor_tensor(
                    out=pw2[:], in0=oh[:], in1=nrt[:].to_broadcast([P, tpb, P]),
                    op=mybir.AluOpType.mult,
                )

                # PE: accumulate sum(w x_src) and sum(w)
                for t in range(tpb):
                    nc.tensor.matmul(
                        out=acc_ps[:, 0:D], lhsT=pw2[:, t, :],
                        rhs=xs_slice(t, t + 1)[:, 0, :],
                        start=(t == 0), stop=False, skip_group_check=True,
                    )
                    nc.tensor.matmul(
                        out=acc_ps[:, D : D + 1], lhsT=pw2[:, t, :],
                        rhs=rct[:, t : t + 1],
                        start=False, stop=(t == tpb - 1), skip_group_check=True,
                    )

                # epilogue: rows = relu(M / s); host scatters block rows
                s_safe = colp.tile([P, 1], F32, tag="ssafe")
                nc.vector.tensor_scalar(
                    out=s_safe[:], in0=acc_ps[:, D : D + 1], scalar1=1e-30,
                    scalar2=None, op0=mybir.AluOpType.max,
                )
                sinv = colp.tile([P, 1], F32, tag="sinv")
                nc.vector.reciprocal(sinv[:], s_safe[:])
                orow = orowp.tile([P, D], F32, tag="orow")
                nc.vector.tensor_scalar(
                    out=orow[:], in0=acc_ps[:, 0:D], scalar1=sinv[:], scalar2=0.0,
                    op0=mybir.AluOpType.mult, op1=mybir.AluOpType.max,
                )
                nc.scalar.dma_start(
                    out=out_ext[b * P : (b + 1) * P, :], in_=orow[:]
                )

    nc.compile()
    return nc


def _wrap16(vals, ncols, pad):
    """[n] -> [128, ncols] int16 in dma_gather's 16-wrapped, 8x-replicated
    partition layout (idx j at [j%16, j//16])."""
    full = np.full(ncols * 16, pad, np.int64)
    full[: len(vals)] = vals
    w = full.reshape(ncols, 16).T.astype(np.int16)  # [16, ncols]
    return np.tile(w, (8, 1))


def _decompose(dst_sorted, src_sorted, N, tpb_lo, tpb_hi, hi_base, max_nodes=P):
    """Blocks of consecutive nodes with <=max_nodes nodes, <=tpb_lo*128
    low-src edges and <=tpb_hi*128 high-src edges."""
    deg = np.bincount(dst_sorted, minlength=N)
    deg_lo = np.bincount(dst_sorted[src_sorted < hi_base], minlength=N)
    deg_hi = deg - deg_lo
    cap_lo, cap_hi = tpb_lo * P, tpb_hi * P
    assert deg_lo.max() <= cap_lo and deg_hi.max() <= cap_hi
    blocks = []
    n0 = e0 = 0
    lo = hi = 0
    for node in range(N):
        dl, dh = int(deg_lo[node]), int(deg_hi[node])
        if (node - n0) >= max_nodes or lo + dl > cap_lo or hi + dh > cap_hi:
            blocks.append((n0, node, e0, e0 + lo + hi))
            n0, e0 = node, e0 + lo + hi
            lo = hi = 0
        lo += dl
        hi += dh
    blocks.append((n0, N, e0, e0 + lo + hi))
    return blocks


def _prep_inputs(x, beta, edge_index, N, D, tpb_lo, tpb_hi, hi_base, ncores):
    tpb = tpb_lo + tpb_hi
    gl, gh = tpb_lo * 8, tpb_hi * 8
    ga = gl + gh
    loop = np.arange(N, dtype=np.int64)
    src = np.concatenate([np.asarray(edge_index[0]), loop]).astype(np.int64)
    dst = np.concatenate([np.asarray(edge_index[1]), loop]).astype(np.int64)
    order = np.argsort(dst, kind="stable")
    src_s = src[order]
    dst_s = dst[order]

    blocks = _decompose(dst_s, src_s, N, tpb_lo, tpb_hi, hi_base)
    nbt = len(blocks)
    sizes = [nbt // ncores + (1 if i < nbt % ncores else 0) for i in range(ncores)]
    NB = max(sizes)

    core_blocks, bpos = [], 0
    for k in range(ncores):
        core_blocks.append(blocks[bpos : bpos + sizes[k]])
        bpos += sizes[k]

    xf32 = np.ascontiguousarray(np.asarray(x), dtype=np.float32)
    norms = np.sqrt(np.maximum((xf32 * xf32).sum(axis=1), 1e-24))
    xn = xf32 / norms[:, None]
    xn_bf = xn.astype(ml_dtypes.bfloat16)
    n_lo = min(hi_base, N)
    n_hi = max(N - hi_base, 8)
    xb_lo = np.ascontiguousarray(xn_bf[:n_lo])
    xb_hi = np.zeros((n_hi, D), ml_dtypes.bfloat16)
    if N > hi_base:
        xb_hi[: N - hi_base] = xn_bf[hi_base:N]
    beta128 = np.full((P, 1), float(np.asarray(beta).reshape(-1)[0]), np.float32)

    in_maps = []
    for k in range(ncores):
        blks = core_blocks[k]
        a_idx = np.zeros((P, NB * ga), np.int16)
        a_db = np.full((NB * tpb, P), BLK_SENTINEL, np.float32)
        a_nm = np.zeros((NB * tpb, P), np.float32)
        a_rc = np.zeros((NB * tpb, P), np.float32)
        a_xdT = np.zeros((NB * P, P), ml_dtypes.bfloat16)  # [cols, d] -> .T later
        for bi, (n0, n1, e0, e1) in enumerate(blks):
            s = src_s[e0:e1]
            d = dst_s[e0:e1]
            lomask = s < hi_base
            s_lo, d_lo = s[lomask], d[lomask]
            s_hi, d_hi = s[~lomask], d[~lomask]
            # sort each half by src for HBM gather locality
            o_lo = np.argsort(s_lo, kind="stable")
            s_lo, d_lo = s_lo[o_lo], d_lo[o_lo]
            o_hi = np.argsort(s_hi, kind="stable")
            s_hi, d_hi = s_hi[o_hi] - hi_base, d_hi[o_hi]
            nlo, nhi = len(s_lo), len(s_hi)
            a_idx[:, bi * ga : bi * ga + gl] = _wrap16(s_lo, gl, 0)
            a_idx[:, bi * ga + gl : (bi + 1) * ga] = _wrap16(s_hi, gh, 0)
            # slot j: tile j//128, partition j%128; lo slots [0, tpb_lo*128)
            dcomb = np.full(tpb * P, BLK_SENTINEL, np.float32)
            dcomb[:nlo] = (d_lo - n0).astype(np.float32)
            dcomb[tpb_lo * P : tpb_lo * P + nhi] = (d_hi - n0).astype(np.float32)
            a_db[bi * tpb : (bi + 1) * tpb] = dcomb.reshape(tpb, P)
            ncomb = np.zeros(tpb * P, np.float32)
            ncomb[:nlo] = norms[s_lo]
            ncomb[tpb_lo * P : tpb_lo * P + nhi] = norms[s_hi + hi_base]
            a_nm[bi * tpb : (bi + 1) * tpb] = ncomb.reshape(tpb, P)
            rcomb = np.zeros(tpb * P, np.float32)
            rcomb[:nlo] = 1.0 / norms[s_lo]
            rcomb[tpb_lo * P : tpb_lo * P + nhi] = 1.0 / norms[s_hi + hi_base]
            a_rc[bi * tpb : (bi + 1) * tpb] = rcomb.reshape(tpb, P)
            a_xdT[bi * P : bi * P + (n1 - n0)] = xn_bf[n0:n1]
        in_maps.append(
            {
                "xb_lo": xb_lo,
                "xb_hi": xb_hi,
                "idx_all": a_idx,
                "dblk": np.ascontiguousarray(a_db.T).astype(ml_dtypes.bfloat16),
                "normt": np.ascontiguousarray(a_nm.T).astype(ml_dtypes.bfloat16),
                "rcol": np.ascontiguousarray(a_rc.T).astype(ml_dtypes.bfloat16),
                "xdT": np.ascontiguousarray(a_xdT.T),
                "beta128": beta128,
            }
        )
    return in_maps, NB, core_blocks


def _enable_axon_ntff():
    """Install the NTFF profile hook that the stub antenv package lacks."""
    import sys, types
    try:
        import antenv

        if "antenv.axon_hooks" not in sys.modules:
            mod = types.ModuleType("antenv.axon_hooks")
            mod._hook = None
            mod.set_axon_ntff_profile_hook = lambda h: setattr(mod, "_hook", h)
            mod.get_axon_ntff_profile_hook = lambda: mod._hook
            sys.modules["antenv.axon_hooks"] = mod
            antenv.axon_hooks = mod
            from trn_agent_boot.trn_boot import _ntff_profile_via_ctypes

            mod._hook = _ntff_profile_via_ctypes("/opt/axon/libaxon_pjrt.so")
        import concourse.bass_utils as bu

        bu.upload_artifacts = lambda tmpdir: tmpdir
        return True
    except Exception as e:
        print(f"ntff hook install failed: {e}")
        return False


def _run(x, beta, edge_index, trace=False):
    from concourse.bass_utils import run_bass_kernel_spmd

    if trace:
        trace = _enable_axon_ntff()
    N, D = x.shape
    in_maps, NB, core_blocks = _prep_inputs(
        x, beta, edge_index, N, D, TPB_LO, TPB_HI, HI_BASE, NCORES
    )
    nc = _build_graph(N, D, NB, TPB_LO, TPB_HI, HI_BASE)
    res = run_bass_kernel_spmd(
        nc, in_maps, core_ids=list(range(NCORES)), trace=trace
    )
    out = np.zeros((N, D), np.float32)
    for k in range(NCORES):
        co = res.results[k]["out"]
        for bi, (n0, n1, e0, e1) in enumerate(core_blocks[k]):
            out[n0:n1] = co[bi * P : bi * P + (n1 - n0)]
    return out, res


def kernel(x, beta, edge_index):
    out, _ = _run(
        np.asarray(x), np.asarray(beta), np.asarray(edge_index), trace=False
    )
    return out


# revision 14
# speedup vs baseline: 3.3139x; 3.3139x over previous
"""AGNN layer (cosine-attention message passing) on 8 TRN2 NeuronCores.

Host sharding: append self-loops, sort edges by destination node, cut the node
range into blocks (<=128 nodes, bounded edge count), hand contiguous block
runs to the 8 cores. Every softmax segment then lives on one core: no
collectives anywhere.

v3 device kernel ("no dst gather"): only the per-edge SRC rows are gathered
(normalized x, bf16, 256B rows — halves SWDGE descriptor+byte traffic vs v1),
one big SWDGE call per table per block. Each gathered group of 8 tiles is
transposed SBUF->SBUF by the DMA XBAR (dma_start_transpose, round-robined
over the sync/scalar/vector HWDGE queues); the Tensor engine then computes
dots[e, n] = xnT_tile^T @ xdT_block against an SBUF-resident per-block
transposed dst table. ACT exponentiates whole groups (scale=beta, constant).
DVE builds the weight matrix in 3 block-wide batched passes:
pw2 = is_equal(iota, dstrel) * exp(beta cos) * ||x_src||, so the two
PSUM-accumulating matmuls per tile yield sum(w * x_src) (norms fold raw x
back out of the normalized rows) and sum(w) (rhs = 1/||x_src||).

Logits are cosines scaled by beta (bounded), so exp never overflows and the
reference's segment-max subtraction cancels exactly -- single pass suffices.
"""

import numpy as np
import ml_dtypes

import concourse.bacc as bacc
import concourse.mybir as mybir
import concourse.tile as tile
import concourse.tile_sem_assignment as _tsa

# Tile's DMASW-lane round-robin is SWDGE-queue-oblivious; a completion sem
# shared by two queues desyncs the ucode's per-queue ring-reclaim waits.
# Partition the 8 lanes so queue q owns lanes {2q, 2q+1}.
_orig_assign_tick = _tsa.TileClockTick._assign_tick


def _assign_tick_queue_aware(self, inst):
    if (
        isinstance(inst, mybir.InstDMAGatherAnt)
        and inst.engine == mybir.EngineType.Pool
    ):
        qn = inst.queue_num
        if not hasattr(self, "_qcnt"):
            self._qcnt = {}
        cnt = self._qcnt.get(qn, 0)
        self._qcnt[qn] = cnt + 1
        self.next_sw_dma_idx = qn * 2 + (cnt % 2)
    return _orig_assign_tick(self, inst)


_tsa.TileClockTick._assign_tick = _assign_tick_queue_aware

P = 128
N_NODES = 50000
D_FEAT = 128
NCORES = 8
HI_BASE = 32768  # int16 gather index limit
TPB_LO = 23  # tiles per block for src<HI_BASE edges
TPB_HI = 12  # tiles per block for src>=HI_BASE edges
GROUP = 8  # tiles per transpose/exp batch
BLK_SENTINEL = 300.0

F32 = mybir.dt.float32
BF16 = mybir.dt.bfloat16
I16 = mybir.dt.int16


def _build_graph(N, D, NB, tpb_lo, tpb_hi, hi_base):
    tpb = tpb_lo + tpb_hi
    gl, gh = tpb_lo * 8, tpb_hi * 8  # idx cols (16-wrapped)
    ga = gl + gh
    nc = bacc.Bacc(
        "TRN2", target_bir_lowering=False, debug=False, enable_asserts=False,
        num_swdge_queues=4,
    )
    n_lo = min(hi_base, N)
    n_hi = max(N - hi_base, 8)
    xb_lo = nc.dram_tensor("xb_lo", [n_lo, D], BF16, kind="ExternalInput").ap()
    xb_hi = nc.dram_tensor("xb_hi", [n_hi, D], BF16, kind="ExternalInput").ap()
    idx_all = nc.dram_tensor("idx_all", [P, NB * ga], I16, kind="ExternalInput").ap()
    dblk = nc.dram_tensor("dblk", [P, NB * tpb], BF16, kind="ExternalInput").ap()
    normt = nc.dram_tensor("normt", [P, NB * tpb], BF16, kind="ExternalInput").ap()
    rcol = nc.dram_tensor("rcol", [P, NB * tpb], BF16, kind="ExternalInput").ap()
    xdT = nc.dram_tensor("xdT", [P, NB * P], BF16, kind="ExternalInput").ap()
    beta128 = nc.dram_tensor("beta128", [P, 1], F32, kind="ExternalInput").ap()
    out_ext = nc.dram_tensor("out", [NB * P, D], F32, kind="ExternalOutput").ap()

    # lo/hi-aligned transpose groups: [t0, t1) ranges that never straddle
    groups = []
    for t0 in range(0, tpb_lo, GROUP):
        groups.append((t0, min(t0 + GROUP, tpb_lo)))
    for t0 in range(tpb_lo, tpb, GROUP):
        groups.append((t0, min(t0 + GROUP, tpb)))

    with tile.TileContext(nc) as tc:
        with (
            tc.tile_pool(name="const", bufs=1) as constp,
            tc.tile_pool(name="idx", bufs=3) as idxp,
            tc.tile_pool(name="meta", bufs=3) as metap,
            tc.tile_pool(name="gsl", bufs=3) as gslp,
            tc.tile_pool(name="gsh", bufs=3) as gshp,
            tc.tile_pool(name="xnt", bufs=2) as xntp,
            tc.tile_pool(name="expd", bufs=2) as expdp,
            tc.tile_pool(name="pw", bufs=2) as pwp,
            tc.tile_pool(name="orow", bufs=2) as orowp,
            tc.tile_pool(name="cols", bufs=4) as colp,
            tc.tile_pool(name="pst", bufs=2, space="PSUM") as pstp,
            tc.tile_pool(name="dots", bufs=2, space="PSUM") as dotsp,
            tc.tile_pool(name="acc", bufs=2, space="PSUM") as accp,
        ):
            # ---- constants ----
            iota_i16 = constp.tile([P, P], I16)
            nc.gpsimd.iota(iota_i16[:], pattern=[[1, P]], base=0, channel_multiplier=0)
            iota_bf = constp.tile([P, P], BF16)
            nc.vector.tensor_copy(iota_bf[:], iota_i16[:])
            iotap_i16 = constp.tile([P, 1], I16)
            nc.gpsimd.iota(iotap_i16[:], pattern=[[1, 1]], base=0, channel_multiplier=1)
            iotap_f = constp.tile([P, 1], F32)
            nc.vector.tensor_copy(iotap_f[:], iotap_i16[:])
            iota_f = constp.tile([P, P], F32)
            nc.vector.tensor_copy(iota_f[:], iota_i16[:])
            ident_bf = constp.tile([P, P], BF16)
            nc.vector.tensor_scalar(
                out=ident_bf[:], in0=iota_f[:], scalar1=iotap_f[:], scalar2=None,
                op0=mybir.AluOpType.is_equal,
            )
            beta_sb = constp.tile([P, 1], F32)
            nc.sync.dma_start(out=beta_sb[:], in_=beta128[:, :])
            # resident per-block transposed dst tables (12.8KB/partition)
            xdT_sb = constp.tile([P, NB * P], BF16)
            nc.sync.dma_start(out=xdT_sb[:], in_=xdT[:, :])

            hw_eng = [nc.sync, nc.scalar]
            self_q = [0, 0]  # swdge queue rr, hwdge engine rr
            for b in range(NB):
                idxt = idxp.tile([P, ga], I16, tag="idxt")
                nc.sync.dma_start(out=idxt[:], in_=idx_all[:, b * ga : (b + 1) * ga])
                dbt = metap.tile([P, tpb, 1], BF16, tag="dbt")
                nc.sync.dma_start(out=dbt[:], in_=dblk[:, b * tpb : (b + 1) * tpb])
                nrt = metap.tile([P, tpb, 1], BF16, tag="nrt")
                nc.sync.dma_start(out=nrt[:], in_=normt[:, b * tpb : (b + 1) * tpb])
                rct = metap.tile([P, tpb], BF16, tag="rct")
                nc.sync.dma_start(out=rct[:], in_=rcol[:, b * tpb : (b + 1) * tpb])

                def gather_rows(out_tile, tab_ap, idx_tile, total):
                    # ucode ring holds 128 in-flight descs/engine -> <=1024 rows
                    off = 0
                    while off < total:
                        ni = min(1024, total - off)
                        nc.gpsimd.dma_gather(
                            out_tile[:, off // P : (off + ni) // P, :],
                            tab_ap,
                            idx_tile[:, off // 16 : (off + ni) // 16],
                            ni, ni, D,
                            queue_num=self_q[0] % 4,
                        )
                        self_q[0] += 1
                        off += ni

                xs_lo = gslp.tile([P, tpb_lo, D], BF16, tag="xsl")
                gather_rows(xs_lo, xb_lo[:, :], idxt[:, 0:gl], tpb_lo * P)
                xs_hi = gshp.tile([P, tpb_hi, D], BF16, tag="xsh")
                gather_rows(xs_hi, xb_hi[:, :], idxt[:, gl:ga], tpb_hi * P)

                def xs_slice(t0, t1):
                    if t1 <= tpb_lo:
                        return xs_lo[:, t0:t1, :]
                    return xs_hi[:, t0 - tpb_lo : t1 - tpb_lo, :]

                acc_ps = accp.tile([P, D + 1], F32, tag="acc")
                xd_blk = xdT_sb[:, b * P : (b + 1) * P]
                xnT = xntp.tile([P, tpb, P], BF16, tag="xnT")
                expd = expdp.tile([P, tpb, P], BF16, tag="expd")

                for (t0, t1) in groups:
                    gsz = t1 - t0
                    # PE: per-tile transposes into a grouped PSUM buffer
                    psT = pstp.tile([P, GROUP, P], BF16, tag="psT")
                    for t in range(t0, t1):
                        nc.tensor.matmul(
                            psT[:, t - t0, :],
                            lhsT=xs_slice(t, t + 1)[:, 0, :], rhs=ident_bf[:],
                            is_transpose=True, start=True, stop=True,
                            skip_group_check=True,
                        )
                    # batched PSUM->SBUF copy, alternating ACT/DVE
                    if self_q[1] % 2 == 0:
                        nc.scalar.activation(
                            out=xnT[:, t0:t1, :], in_=psT[:, 0:gsz, :],
                            func=mybir.ActivationFunctionType.Copy,
                        )
                    else:
                        nc.vector.tensor_copy(xnT[:, t0:t1, :], psT[:, 0:gsz, :])
                    self_q[1] += 1
                    # PE: dots[e, n] over the block's node range
                    dots = dotsp.tile([P, GROUP, P], F32, tag="dots")
                    for t in range(t0, t1):
                        nc.tensor.matmul(
                            dots[:, t - t0, :], lhsT=xnT[:, t, :], rhs=xd_blk,
                            start=True, stop=True, skip_group_check=True,
                        )
                    # ACT: batched exp(beta * cos)
                    nc.scalar.activation(
                        out=expd[:, t0:t1, :], in_=dots[:, 0:gsz, :],
                        func=mybir.ActivationFunctionType.Exp, scale=beta_sb[:],
                    )

                # DVE: block-wide batched weight build
                oh = pwp.tile([P, tpb, P], BF16, tag="oh")
                nc.vector.tensor_tensor(
                    out=oh[:], in0=iota_bf[:][:, None, :].to_broadcast([P, tpb, P]),
                    in1=dbt[:].to_broadcast([P, tpb, P]),
                    op=mybir.AluOpType.is_equal,
                )
                nc.vector.tensor_tensor(
                    out=oh[:], in0=oh[:], in1=expd[:],
                    op=mybir.AluOpType.mult,
                )
                pw2 = pwp.tile([P, tpb, P], BF16, tag="pw2")
                nc.vector.tensor_tensor(
                    out=pw2[:], in0=oh[:], in1=nrt[:].to_broadcast([P, tpb, P]),
                    op=mybir.AluOpType.mult,
                )

                # PE: accumulate sum(w x_src) and sum(w)
                for t in range(tpb):
                    nc.tensor.matmul(
                        out=acc_ps[:, 0:D], lhsT=pw2[:, t, :],
                        rhs=xs_slice(t, t + 1)[:, 0, :],
                        start=(t == 0), stop=False, skip_group_check=True,
                    )
                    nc.tensor.matmul(
                        out=acc_ps[:, D : D + 1], lhsT=pw2[:, t, :],
                        rhs=rct[:, t : t + 1],
                        start=False, stop=(t == tpb - 1), skip_group_check=True,
                    )

                # epilogue: rows = relu(M / s); host scatters block rows
                s_safe = colp.tile([P, 1], F32, tag="ssafe")
                nc.vector.tensor_scalar(
                    out=s_safe[:], in0=acc_ps[:, D : D + 1], scalar1=1e-30,
                    scalar2=None, op0=mybir.AluOpType.max,
                )
                sinv = colp.tile([P, 1], F32, tag="sinv")
                nc.vector.reciprocal(sinv[:], s_safe[:])
                orow = orowp.tile([P, D], F32, tag="orow")
                nc.vector.tensor_scalar(
                    out=orow[:], in0=acc_ps[:, 0:D], scalar1=sinv[:], scalar2=0.0,
                    op0=mybir.AluOpType.mult, op1=mybir.AluOpType.max,
                )
                nc.scalar.dma_start(
                    out=out_ext[b * P : (b + 1) * P, :], in_=orow[:]
                )

    nc.compile()
    return nc


def _wrap16(vals, ncols, pad):
    """[n] -> [128, ncols] int16 in dma_gather's 16-wrapped, 8x-replicated
    partition layout (idx j at [j%16, j//16])."""
    full = np.full(ncols * 16, pad, np.int64)
    full[: len(vals)] = vals
    w = full.reshape(ncols, 16).T.astype(np.int16)  # [16, ncols]
    return np.tile(w, (8, 1))


def _decompose(dst_sorted, src_sorted, N, tpb_lo, tpb_hi, hi_base, max_nodes=P):
    """Blocks of consecutive nodes with <=max_nodes nodes, <=tpb_lo*128
    low-src edges and <=tpb_hi*128 high-src edges."""
    deg = np.bincount(dst_sorted, minlength=N)
    deg_lo = np.bincount(dst_sorted[src_sorted < hi_base], minlength=N)
    deg_hi = deg - deg_lo
    cap_lo, cap_hi = tpb_lo * P, tpb_hi * P
    assert deg_lo.max() <= cap_lo and deg_hi.max() <= cap_hi
    blocks = []
    n0 = e0 = 0
    lo = hi = 0
    for node in range(N):
        dl, dh = int(deg_lo[node]), int(deg_hi[node])
        if (node - n0) >= max_nodes or lo + dl > cap_lo or hi + dh > cap_hi:
            blocks.append((n0, node, e0, e0 + lo + hi))
            n0, e0 = node, e0 + lo + hi
            lo = hi = 0
        lo += dl
        hi += dh
    blocks.append((n0, N, e0, e0 + lo + hi))
    return blocks


def _prep_inputs(x, beta, edge_index, N, D, tpb_lo, tpb_hi, hi_base, ncores):
    tpb = tpb_lo + tpb_hi
    gl, gh = tpb_lo * 8, tpb_hi * 8
    ga = gl + gh
    loop = np.arange(N, dtype=np.int64)
    src = np.concatenate([np.asarray(edge_index[0]), loop]).astype(np.int64)
    dst = np.concatenate([np.asarray(edge_index[1]), loop]).astype(np.int64)
    order = np.argsort(dst, kind="stable")
    src_s = src[order]
    dst_s = dst[order]

    blocks = _decompose(dst_s, src_s, N, tpb_lo, tpb_hi, hi_base)
    nbt = len(blocks)
    sizes = [nbt // ncores + (1 if i < nbt % ncores else 0) for i in range(ncores)]
    NB = max(sizes)

    core_blocks, bpos = [], 0
    for k in range(ncores):
        core_blocks.append(blocks[bpos : bpos + sizes[k]])
        bpos += sizes[k]

    xf32 = np.ascontiguousarray(np.asarray(x), dtype=np.float32)
    norms = np.sqrt(np.maximum((xf32 * xf32).sum(axis=1), 1e-24))
    xn = xf32 / norms[:, None]
    xn_bf = xn.astype(ml_dtypes.bfloat16)
    n_lo = min(hi_base, N)
    n_hi = max(N - hi_base, 8)
    xb_lo = np.ascontiguousarray(xn_bf[:n_lo])
    xb_hi = np.zeros((n_hi, D), ml_dtypes.bfloat16)
    if N > hi_base:
        xb_hi[: N - hi_base] = xn_bf[hi_base:N]
    beta128 = np.full((P, 1), float(np.asarray(beta).reshape(-1)[0]), np.float32)

    in_maps = []
    for k in range(ncores):
        blks = core_blocks[k]
        a_idx = np.zeros((P, NB * ga), np.int16)
        a_db = np.full((NB * tpb, P), BLK_SENTINEL, np.float32)
        a_nm = np.zeros((NB * tpb, P), np.float32)
        a_rc = np.zeros((NB * tpb, P), np.float32)
        a_xdT = np.zeros((NB * P, P), ml_dtypes.bfloat16)  # [cols, d] -> .T later
        for bi, (n0, n1, e0, e1) in enumerate(blks):
            s = src_s[e0:e1]
            d = dst_s[e0:e1]
            lomask = s < hi_base
            s_lo, d_lo = s[lomask], d[lomask]
            s_hi, d_hi = s[~lomask], d[~lomask]
            # sort each half by src for HBM gather locality
            o_lo = np.argsort(s_lo, kind="stable")
            s_lo, d_lo = s_lo[o_lo], d_lo[o_lo]
            o_hi = np.argsort(s_hi, kind="stable")
            s_hi, d_hi = s_hi[o_hi] - hi_base, d_hi[o_hi]
            nlo, nhi = len(s_lo), len(s_hi)
            a_idx[:, bi * ga : bi * ga + gl] = _wrap16(s_lo, gl, 0)
            a_idx[:, bi * ga + gl : (bi + 1) * ga] = _wrap16(s_hi, gh, 0)
            # slot j: tile j//128, partition j%128; lo slots [0, tpb_lo*128)
            dcomb = np.full(tpb * P, BLK_SENTINEL, np.float32)
            dcomb[:nlo] = (d_lo - n0).astype(np.float32)
            dcomb[tpb_lo * P : tpb_lo * P + nhi] = (d_hi - n0).astype(np.float32)
            a_db[bi * tpb : (bi + 1) * tpb] = dcomb.reshape(tpb, P)
            ncomb = np.zeros(tpb * P, np.float32)
            ncomb[:nlo] = norms[s_lo]
            ncomb[tpb_lo * P : tpb_lo * P + nhi] = norms[s_hi + hi_base]
            a_nm[bi * tpb : (bi + 1) * tpb] = ncomb.reshape(tpb, P)
            rcomb = np.zeros(tpb * P, np.float32)
            rcomb[:nlo] = 1.0 / norms[s_lo]
            rcomb[tpb_lo * P : tpb_lo * P + nhi] = 1.0 / norms[s_hi + hi_base]
            a_rc[bi * tpb : (bi + 1) * tpb] = rcomb.reshape(tpb, P)
            a_xdT[bi * P : bi * P + (n1 - n0)] = xn_bf[n0:n1]
        in_maps.append(
            {
                "xb_lo": xb_lo,
                "xb_hi": xb_hi,
                "idx_all": a_idx,
                "dblk": np.ascontiguousarray(a_db.T).astype(ml_dtypes.bfloat16),
                "normt": np.ascontiguousarray(a_nm.T).astype(ml_dtypes.bfloat16),
                "rcol": np.ascontiguousarray(a_rc.T).astype(ml_dtypes.bfloat16),
                "xdT": np.ascontiguousarray(a_xdT.T),
                "beta128": beta128,
            }
        )
    return in_maps, NB, core_blocks


def _enable_axon_ntff():
    """Install the NTFF profile hook that the stub antenv package lacks."""
    import sys, types
    try:
        import antenv

        if "antenv.axon_hooks" not in sys.modules:
            mod = types.ModuleType("antenv.axon_hooks")
            mod._hook = None
            mod.set_axon_ntff_profile_hook = lambda h: setattr(mod, "_hook", h)
            mod.get_axon_ntff_profile_hook = lambda: mod._hook
            sys.modules["antenv.axon_hooks"] = mod
            antenv.axon_hooks = mod
            from trn_agent_boot.trn_boot import _ntff_profile_via_ctypes

            mod._hook = _ntff_profile_via_ctypes("/opt/axon/libaxon_pjrt.so")
        import concourse.bass_utils as bu

        bu.upload_artifacts = lambda tmpdir: tmpdir
        return True
    except Exception as e:
        print(f"ntff hook install failed: {e}")
        return False


def _run(x, beta, edge_index, trace=False):
    from concourse.bass_utils import run_bass_kernel_spmd

    if trace:
        trace = _enable_axon_ntff()
    N, D = x.shape
    in_maps, NB, core_blocks = _prep_inputs(
        x, beta, edge_index, N, D, TPB_LO, TPB_HI, HI_BASE, NCORES
    )
    nc = _build_graph(N, D, NB, TPB_LO, TPB_HI, HI_BASE)
    res = run_bass_kernel_spmd(
        nc, in_maps, core_ids=list(range(NCORES)), trace=trace
    )
    out = np.zeros((N, D), np.float32)
    for k in range(NCORES):
        co = res.results[k]["out"]
        for bi, (n0, n1, e0, e1) in enumerate(core_blocks[k]):
            out[n0:n1] = co[bi * P : bi * P + (n1 - n0)]
    return out, res


def kernel(x, beta, edge_index):
    out, _ = _run(
        np.asarray(x), np.asarray(beta), np.asarray(edge_index), trace=False
    )
    return out
